# revision 10
# baseline (speedup 1.0000x reference)
"""AttentionBlock (GroupNorm + 8-head self-attention + proj + residual) on
8 TRN2 NeuronCores, data-parallel over batch (16 batches -> 2 per core).

Self-contained: builds the Bass graph, shards + preprocesses inputs host-side
(weight transposition / head-pair permutation), runs SPMD on cores 0-7,
gathers the full output.
"""
import sys

if "/opt/trn_rl_repo" not in sys.path:
    sys.path.insert(0, "/opt/trn_rl_repo")

import numpy as np

import concourse.bass as bass
import concourse.tile as tile
from concourse import bacc, mybir
from concourse.bass_utils import run_bass_kernel_spmd

F32 = mybir.dt.float32
BF16 = mybir.dt.bfloat16
AF = mybir.ActivationFunctionType
OP = mybir.AluOpType

B, C, H, W = 16, 512, 32, 32
T = H * W                      # 1024
HEADS = 8
CH = C // HEADS                # 64 per-head channels
GROUPS = 32
GSIZE = C // GROUPS            # 16 channels per group
EPS = 1e-5
NCORES = 8
BPC = B // NCORES              # batches per core
P = 128
KT = C // P                    # 4 k-tiles over the 512 input channels
SCALE = 1.0 / np.sqrt(CH)      # (q*s)·(k*s) = qk/8, applied inside exp


def build_nc():
    nc = bacc.Bacc(None, target_bir_lowering=False, debug=False)

    x_ext = nc.declare_dram_parameter("x", [BPC, C, T], F32, isOutput=False)
    gns_ext = nc.declare_dram_parameter("gn_scale", [C], F32, isOutput=False)
    gnb_ext = nc.declare_dram_parameter("gn_bias", [C], F32, isOutput=False)
    # host-preprocessed weights (transposed / head-pair permuted):
    # wqkT: [inc 512, outc' 1024], outc' = 128*mt + p; mt<4 -> q of head
    #   2*mt + p//64 row p%64 ; mt>=4 -> k of head 2*(mt-4)+p//64
    # wvT:  [inc 512, c'' 512], c'' = 64h + r (v rows)
    # wprojT: [inc(c'') 512, outc 512]
    wqkT_ext = nc.declare_dram_parameter("wqkT", [C, 2 * C], F32, isOutput=False)
    wvT_ext = nc.declare_dram_parameter("wvT", [C, C], F32, isOutput=False)
    wprojT_ext = nc.declare_dram_parameter("wprojT", [C, C], F32, isOutput=False)
    bqk_ext = nc.declare_dram_parameter("bqk", [P, 8], F32, isOutput=False)
    bv_ext = nc.declare_dram_parameter("bv", [C], F32, isOutput=False)
    bproj_ext = nc.declare_dram_parameter("b_proj", [C], F32, isOutput=False)
    gmat_ext = nc.declare_dram_parameter("gmat", [P, 8], F32, isOutput=False)
    gmatT_ext = nc.declare_dram_parameter("gmatT", [8, P], F32, isOutput=False)
    out_ext = nc.declare_dram_parameter("out", [BPC, C, T], F32, isOutput=True)

    with tile.TileContext(nc) as tc:
        import contextlib
        with contextlib.ExitStack() as ctx:
            _build_body(ctx, tc, nc, x_ext, gns_ext, gnb_ext, wqkT_ext, wvT_ext,
                        wprojT_ext, bqk_ext, bv_ext, bproj_ext, gmat_ext,
                        gmatT_ext, out_ext)
    nc.compile()
    return nc


def _build_body(ctx, tc, nc, x_ext, gns_ext, gnb_ext, wqkT_ext, wvT_ext,
                wprojT_ext, bqk_ext, bv_ext, bproj_ext, gmat_ext, gmatT_ext,
                out_ext):
    # pools
    big32 = ctx.enter_context(tc.tile_pool(name="big32", bufs=3))   # 16KB/par
    bbf = ctx.enter_context(tc.tile_pool(name="bbf", bufs=2))
    hpool = ctx.enter_context(tc.tile_pool(name="hpool", bufs=2))
    vtp = ctx.enter_context(tc.tile_pool(name="vtp", bufs=2))
    espool = ctx.enter_context(tc.tile_pool(name="espool", bufs=3))
    rpool = ctx.enter_context(tc.tile_pool(name="rpool", bufs=2))
    dpool = ctx.enter_context(tc.tile_pool(name="dpool", bufs=2))
    singles = ctx.enter_context(tc.tile_pool(name="singles", bufs=1))
    small = ctx.enter_context(tc.tile_pool(name="small", bufs=4))
    stp = ctx.enter_context(tc.tile_pool(name="stp", bufs=2, space="PSUM"))
    pvp = ctx.enter_context(tc.tile_pool(name="pvp", bufs=1, space="PSUM"))
    mmp = ctx.enter_context(tc.tile_pool(name="mmp", bufs=2, space="PSUM"))

    # ---------------- one-time: constants + weight load/cast ----------------
    gns_sb = singles.tile([P, KT], F32)
    nc.sync.dma_start(out=gns_sb[:], in_=gns_ext.rearrange("(o p) -> p o", p=P))
    gnb_sb = singles.tile([P, KT], F32)
    nc.sync.dma_start(out=gnb_sb[:], in_=gnb_ext.rearrange("(o p) -> p o", p=P))
    bproj_sb = singles.tile([P, KT], F32)
    nc.sync.dma_start(out=bproj_sb[:], in_=bproj_ext.rearrange("(o p) -> p o", p=P))
    bqk_sb = singles.tile([P, 8], F32)
    nc.sync.dma_start(out=bqk_sb[:], in_=bqk_ext[:])
    bv_bc = singles.tile([P, C], F32)
    nc.sync.dma_start(out=bv_bc[0:1, :], in_=bv_ext[:].unsqueeze(0))
    nc.gpsimd.partition_broadcast(bv_bc[:], bv_bc[0:1, :])
    g16 = singles.tile([P, 8], F32)
    nc.sync.dma_start(out=g16[:], in_=gmat_ext[:])
    gT1 = singles.tile([8, P], F32)
    nc.sync.dma_start(out=gT1[:], in_=gmatT_ext[:])
    eps8 = singles.tile([8, 1], F32)
    nc.vector.memset(eps8[:], EPS)

    # weights: fp32 stage (big32 pool) -> bf16 persistent
    wqkT = singles.tile([P, KT, 2 * C], BF16)
    wvT = singles.tile([P, KT, C], BF16)
    wprojT = singles.tile([P, KT, C], BF16)
    wqk_f32 = big32.tile([P, KT, 2 * C], F32, tag="big32")
    nc.sync.dma_start(out=wqk_f32[:],
                      in_=wqkT_ext.rearrange("(ki p) o -> p ki o", p=P))
    nc.vector.tensor_copy(wqkT[:], wqk_f32[:])
    wvp_f32 = big32.tile([P, 2, KT, C], F32, tag="big32")
    nc.sync.dma_start(out=wvp_f32[:, 0],
                      in_=wvT_ext.rearrange("(ki p) o -> p ki o", p=P))
    nc.sync.dma_start(out=wvp_f32[:, 1],
                      in_=wprojT_ext.rearrange("(ki p) o -> p ki o", p=P))
    nc.vector.tensor_copy(wvT[:], wvp_f32[:, 0])
    nc.vector.tensor_copy(wprojT[:], wvp_f32[:, 1])

    # ---------------- per-batch pipeline ----------------
    for b in range(BPC):
        x_sb = big32.tile([P, KT, T], F32, tag="big32")
        for o in range(KT):
            nc.sync.dma_start(out=x_sb[:, o, :], in_=x_ext[b, o * P:(o + 1) * P, :])

        # ---- GroupNorm -> h (bf16) ----
        h_sb = hpool.tile([P, KT, T], BF16)
        for o in range(KT):
            stats = small.tile([P, 2, nc.vector.BN_STATS_DIM], F32, tag="bnstats")
            nc.vector.bn_stats(out=stats[:, 0, :], in_=x_sb[:, o, 0:512])
            nc.vector.bn_stats(out=stats[:, 1, :], in_=x_sb[:, o, 512:1024])
            mv = small.tile([P, 2], F32, tag="mv")
            nc.vector.bn_aggr(out=mv[:], in_=stats[:])
            st2 = small.tile([P, 2], F32, tag="st2")   # (mean, E[x^2])
            nc.vector.tensor_copy(st2[:, 0:1], mv[:, 0:1])
            nc.vector.tensor_tensor(st2[:, 1:2], mv[:, 0:1], mv[:, 0:1], OP.mult)
            nc.vector.tensor_add(st2[:, 1:2], st2[:, 1:2], mv[:, 1:2])
            gp = mmp.tile([P, 512], F32, tag="mm")
            nc.tensor.matmul(gp[0:8, 0:2], g16[:], st2[:], start=True, stop=True)
            gsb = small.tile([8, 2], F32, tag="gsb")   # (mu_g, E_g)
            nc.vector.tensor_copy(gsb[:], gp[0:8, 0:2])
            gmu = small.tile([8, 2], F32, tag="gmu")   # (mu_g, rstd_g)
            nc.vector.tensor_copy(gmu[:, 0:1], gsb[:, 0:1])
            var = small.tile([8, 1], F32, tag="gvar")
            nc.vector.tensor_tensor(var[:], gsb[:, 0:1], gsb[:, 0:1], OP.mult)
            nc.vector.tensor_sub(var[:], gsb[:, 1:2], var[:])
            nc.scalar.activation(var[:], var[:], AF.Sqrt, bias=eps8[:])
            nc.vector.reciprocal(gmu[:, 1:2], var[:])
            bp = mmp.tile([P, 512], F32, tag="mm")
            nc.tensor.matmul(bp[0:P, 0:2], gT1[:], gmu[:], start=True, stop=True)
            ab = small.tile([P, 2], F32, tag="ab")     # (a, b): h = a*x + b
            nc.vector.tensor_tensor(ab[:, 0:1], bp[:, 1:2], gns_sb[:, o:o + 1],
                                    OP.mult)
            nc.vector.tensor_tensor(ab[:, 1:2], bp[:, 0:1], ab[:, 0:1], OP.mult)
            nc.vector.tensor_sub(ab[:, 1:2], gnb_sb[:, o:o + 1], ab[:, 1:2])
            nc.vector.tensor_scalar(h_sb[:, o, :], x_sb[:, o, :],
                                    ab[:, 0:1], ab[:, 1:2], OP.mult, OP.add)

        # ---- QKV ----
        # qk_sb: [128, 8, 1024]; tile j<4 = [q_{2j}; q_{2j+1}], 4+j = k pairs
        qk_sb = bbf.tile([P, 8, T], BF16, tag="bbf")
        for mt in range(8):
            for n in range(2):
                ps = mmp.tile([P, 512], F32, tag="mm")
                for ki in range(KT):
                    nc.tensor.matmul(ps[:], wqkT[:, ki, mt * P:(mt + 1) * P],
                                     h_sb[:, ki, n * 512:(n + 1) * 512],
                                     start=(ki == 0), stop=(ki == KT - 1))
                nc.vector.tensor_scalar_add(qk_sb[:, mt, n * 512:(n + 1) * 512],
                                            ps[:], bqk_sb[:, mt:mt + 1])

        # vT_sb: [128 (s), 8 (s-blk), 8*65]; head h = [v(64) | one] at 65h
        vT_sb = vtp.tile([P, 8, 8 * 65], BF16)
        vv = vT_sb.rearrange("p s (h w) -> p s h w", w=65)
        nc.vector.memset(vv[:, :, :, 64], 1.0)
        for sb in range(8):
            ps = mmp.tile([P, 512], F32, tag="mm")
            for ki in range(KT):
                nc.tensor.matmul(ps[:], h_sb[:, ki, sb * P:(sb + 1) * P],
                                 wvT[:, ki, :],
                                 start=(ki == 0), stop=(ki == KT - 1))
            pv_ = ps.rearrange("p (h z) -> p h z", z=CH)
            bvv = bv_bc.rearrange("p (h z) -> p h z", z=CH)
            nc.vector.tensor_tensor(vv[:, sb, :, 0:64], pv_[:], bvv[:], OP.add)

        # ---- attention per head ----
        r_sb = rpool.tile([P, KT, T], BF16)
        for h in range(HEADS):
            par = 64 * (h % 2)
            q_h = qk_sb[par:par + 64, h // 2, :]
            k_h = qk_sb[par:par + 64, 4 + h // 2, :]
            pv = pvp.tile([P, T], F32, tag="pv")
            for sb in range(8):
                st = stp.tile([P, T], F32, tag="st")
                es = espool.tile([P, T], BF16)
                for n in range(2):
                    nc.tensor.matmul(st[:, n * 512:(n + 1) * 512],
                                     k_h[:, sb * P:(sb + 1) * P],
                                     q_h[:, n * 512:(n + 1) * 512],
                                     start=True, stop=True)
                # exp(S/8) -> bf16 (no max subtraction: |S/8| is small)
                nc.scalar.activation(es[:], st[:], AF.Exp, scale=float(SCALE))
                lhsT = vT_sb[:, sb, 65 * h: 65 * h + 65]
                for n in range(2):
                    nc.tensor.matmul(pv[0:65, n * 512:(n + 1) * 512], lhsT,
                                     es[:, n * 512:(n + 1) * 512],
                                     start=(sb == 0), stop=(sb == 7))
            # denom at psum row 64; v rows 0..63
            dinv = dpool.tile([1, T], F32, tag="dinv")
            nc.vector.reciprocal(dinv[:], pv[64:65, :])
            dbc = dpool.tile([64, T], F32, tag="dbc")
            nc.gpsimd.partition_broadcast(dbc[:], dinv[:])
            # r[c''] = pv_v * dinv ; odd heads write partitions 64..127
            nc.vector.tensor_tensor(
                r_sb[par:par + 64, h // 2, :],
                pv[0:64, :], dbc[:], OP.mult)

        # ---- proj + residual ----
        out_sb = big32.tile([P, KT, T], F32, tag="big32")
        for mo in range(KT):
            for n in range(2):
                ps = mmp.tile([P, 512], F32, tag="mm")
                for ki in range(KT):
                    nc.tensor.matmul(ps[:], wprojT[:, ki, mo * P:(mo + 1) * P],
                                     r_sb[:, ki, n * 512:(n + 1) * 512],
                                     start=(ki == 0), stop=(ki == KT - 1))
                nc.vector.tensor_scalar_add(ps[:], ps[:], bproj_sb[:, mo:mo + 1])
                nc.vector.tensor_add(out_sb[:, mo, n * 512:(n + 1) * 512],
                                     ps[:], x_sb[:, mo, n * 512:(n + 1) * 512])
        for o in range(KT):
            nc.sync.dma_start(out=out_ext[b, o * P:(o + 1) * P, :],
                              in_=out_sb[:, o, :])


def host_prep(gn_scale, gn_bias, w_qkv, b_qkv, w_proj, b_proj):
    """Transpose/permute weights host-side into the kernel's layouts."""
    w_qkv = np.asarray(w_qkv, np.float32).reshape(HEADS, 3, CH, C)
    b_qkv = np.asarray(b_qkv, np.float32).reshape(HEADS, 3, CH)
    wq = w_qkv[:, 0]          # [h, r, inc]
    wk = w_qkv[:, 1]
    wv = w_qkv[:, 2]
    # outc' = 128*mt + p; mt<4: q head 2mt+p//64; mt>=4: k head 2(mt-4)+p//64
    wqkT = np.empty((C, 2 * C), np.float32)
    bqk = np.empty((P, 8), np.float32)
    for mt in range(4):
        for u in range(2):
            h = 2 * mt + u
            wqkT[:, 128 * mt + 64 * u: 128 * mt + 64 * u + 64] = wq[h].T
            wqkT[:, 512 + 128 * mt + 64 * u: 512 + 128 * mt + 64 * u + 64] = wk[h].T
            bqk[64 * u:64 * u + 64, mt] = b_qkv[h, 0]
            bqk[64 * u:64 * u + 64, 4 + mt] = b_qkv[h, 1]
    wvT = np.ascontiguousarray(wv.reshape(C, C).T)          # [inc, c'']
    bv = np.ascontiguousarray(b_qkv[:, 2].reshape(C))
    wprojT = np.ascontiguousarray(np.asarray(w_proj, np.float32).T)
    g16 = np.zeros((P, 8), np.float32)
    gT1 = np.zeros((8, P), np.float32)
    for g in range(8):
        g16[g * 16:(g + 1) * 16, g] = 1.0 / GSIZE
        gT1[g, g * 16:(g + 1) * 16] = 1.0
    return {
        "gn_scale": np.asarray(gn_scale, np.float32),
        "gn_bias": np.asarray(gn_bias, np.float32),
        "wqkT": np.ascontiguousarray(wqkT),
        "wvT": wvT,
        "wprojT": wprojT,
        "bqk": np.ascontiguousarray(bqk),
        "bv": bv,
        "b_proj": np.asarray(b_proj, np.float32),
        "gmat": g16,
        "gmatT": gT1,
    }


_NC = None


def _get_nc():
    global _NC
    if _NC is None:
        _NC = build_nc()
    return _NC


def kernel(x, gn_scale, gn_bias, w_qkv, b_qkv, w_proj, b_proj):
    x = np.ascontiguousarray(np.asarray(x, dtype=np.float32))
    nc = _get_nc()
    xf = x.reshape(B, C, T)
    shared = host_prep(gn_scale, gn_bias, w_qkv, b_qkv, w_proj, b_proj)
    in_maps = [
        {"x": np.ascontiguousarray(xf[i * BPC:(i + 1) * BPC]), **shared}
        for i in range(NCORES)
    ]
    res = run_bass_kernel_spmd(nc, in_maps, core_ids=list(range(NCORES)))
    out = np.concatenate([np.asarray(res.results[i]["out"]) for i in range(NCORES)],
                         axis=0)
    return out.reshape(B, C, H, W).astype(np.float32)
